# revision 3
# baseline (speedup 1.0000x reference)
"""Trainium2 Bass kernel for nn_MultiHeadCrossAttention_57638461112647.

Sharding: 8 cores = 2 batches x 4-way split over attention *keys* (and,
identically, over output tokens). The softmax in the reference is over the
*query* axis (axis=1), so with scores laid out (keys on partitions, queries on
free) the softmax denominator Z[k] is a free-axis row-sum, fully core-local.
The only cross-core exchange is one ReduceScatter of the attention output
partials x^T = V'^T E (48 x 4096) within each 4-core batch group.

Host-side prep (cheap, O(C^2) / O(N*C)): fold BN-as-affine + biases into the
1x1-conv weights, fold the attention scale into Wq, fold the 3x3x3-conv bias
through the following 1x1 conv, add the (constant) 3D positional encodings,
pad the conv input with its halo, and slice per-core chunks.

Device pipeline per core (channel-major layouts, channels on partitions):
  Y1 = relu(WyF @ (Y+pe))          -> Q^T = Wq'^T Y1 (48x4096, all queries)
  Y1k/S1k from pre-sliced chunks   -> K^T (48x1024), V (1024x48, own keys)
  scores^T = K^T^T Q^T             (PE, 8 k-tiles of 128 x 4096 q)
  E = exp(scores), Z = rowsum      (ACT, accum_out fused; no max-sub needed:
                                    scores are O(0.2))
  V' = V / Z (bf16), E bf16        -> x^T partial = sum_kt V'[kt]^T E[kt] (PE)
  ReduceScatter(4-core group)      -> x^T reduced for own 1024 tokens
  out  rows 0-47: relu(WoF @ x^T + b) * (S+pe)[own chunk]
  conv3x3x3 as 27 accumulating matmuls on a padded slab (runs during the RS)
  out rows 48-95: relu(Wy2F @ conv3 + b')
"""
import numpy as np
import ml_dtypes

import concourse.bass as bass
import concourse.mybir as mybir
import concourse.tile as tile
from concourse import bacc
from concourse.bass_utils import run_bass_kernel_spmd

F32 = mybir.dt.float32
BF16 = mybir.dt.bfloat16
AF = mybir.ActivationFunctionType

B, Cy, Cs, D, H, W = 2, 96, 48, 16, 16, 16
N = D * H * W            # 4096 tokens
NC = 8                   # cores
G = 4                    # cores per batch
KC = N // G              # keys / output tokens per core = 1024
KT = KC // 128           # k-tiles per core = 8
QC = 512                 # q (free-dim) chunk per matmul
EPS = 1e-5

_cache = {}


# ---------------------------------------------------------------- host prep
def _pe3d(C, x, y, z):
    """Transcription of reference.pe3d (incl. its quirky torch broadcasting)."""
    c = int(np.ceil(C / 3))
    inv_freq = (1.0 / (10000.0 ** (np.arange(0, c, 2, dtype=np.float32) / c))
                ).astype(np.float32)

    def emb(n):
        s = np.arange(n, dtype=np.float32)[:, None] * inv_freq[None, :]
        return np.concatenate([np.sin(s), np.cos(s)], axis=-1).astype(np.float32)

    out = np.zeros((x, y, z, 3 * c), np.float32)
    out[..., :c] = emb(x)[:, None, :]        # broadcasts against (y, z, c)
    out[..., c:2 * c] = emb(y)[:, None, :]
    out[..., 2 * c:3 * c] = emb(z)
    return np.ascontiguousarray(out[..., :C].transpose(3, 0, 1, 2))  # (C,x,y,z)


def _prepare(inputs):
    f = lambda a: np.ascontiguousarray(np.asarray(a, np.float32))
    Y, S = f(inputs['Y']), f(inputs['S'])

    pe_s = _pe3d(Cs, D, H, W).reshape(Cs, N)
    pe_y = _pe3d(Cy, D, H, W).reshape(Cy, N)
    Scm = S.reshape(B, Cs, N) + pe_s[None]
    Ycm = Y.reshape(B, Cy, N) + pe_y[None]

    sb = lambda g: f(g) / np.sqrt(np.float32(1.0) + np.float32(EPS))

    def fold(w, b, g, be):
        s = sb(g)
        return f(w) * s[:, None], (f(b) * s + f(be)).astype(np.float32)

    WsF, bsF = fold(inputs['w_s'], inputs['b_s'], inputs['g_s'], inputs['be_s'])
    WyF, byF = fold(inputs['w_y'], inputs['b_y'], inputs['g_y'], inputs['be_y'])
    WoF, boF = fold(inputs['w_o'], inputs['b_o'], inputs['g_o'], inputs['be_o'])
    Wy2F, by2F = fold(inputs['w_y2'], inputs['b_y2'], inputs['g_y2'], inputs['be_y2'])
    by2FF = (Wy2F @ f(inputs['b3']) + by2F).astype(np.float32)

    c = np.ascontiguousarray
    Wq = c(f(inputs['Wq']) * np.float32(Cs) ** np.float32(-0.5))
    Wk = c(f(inputs['Wk']))
    Wv = c(f(inputs['Wv']))
    WsF_T, WyF_T = c(WsF.T), c(WyF.T)
    WoF_T, Wy2F_T = c(WoF.T), c(Wy2F.T)
    w3T = c(f(inputs['w3']).reshape(Cy, Cy, 27).transpose(2, 1, 0))  # (27,96,96)
    BIAS = c(np.stack([byF, bsF, boF, by2FF], axis=1))               # (48,4)

    Ypad = np.zeros((B, Cy, D + 2, H + 2, W + 2), np.float32)
    Ypad[:, :, 1:-1, 1:-1, 1:-1] = Ycm.reshape(B, Cy, D, H, W)

    in_maps = []
    for core in range(NC):
        b, g = divmod(core, G)
        d0 = g * (D // G)
        in_maps.append(dict(
            Yf=c(Ycm[b]),                                  # (96,4096)
            Yk=c(Ycm[b, :, g * KC:(g + 1) * KC]),          # (96,1024)
            Sk=c(Scm[b, :, g * KC:(g + 1) * KC]),          # (48,1024)
            Yslab=c(Ypad[b, :, d0:d0 + 6, :, :]),          # (96,6,18,18)
            WsF_T=WsF_T, WyF_T=WyF_T, WoF_T=WoF_T, Wy2F_T=Wy2F_T,
            Wq=Wq, Wk=Wk, Wv=Wv, w3T=w3T, BIAS=BIAS,
        ))
    return in_maps


# ---------------------------------------------------------------- bass build
def _build(repeat=1):
    nc = bacc.Bacc("TRN2", target_bir_lowering=False, debug=False, num_devices=NC)

    Yf = nc.dram_tensor("Yf", [Cy, N], F32, kind="ExternalInput")
    Yk = nc.dram_tensor("Yk", [Cy, KC], F32, kind="ExternalInput")
    Sk = nc.dram_tensor("Sk", [Cs, KC], F32, kind="ExternalInput")
    Yslab = nc.dram_tensor("Yslab", [Cy, 6, 18, 18], F32, kind="ExternalInput")
    WsF_T = nc.dram_tensor("WsF_T", [Cs, Cs], F32, kind="ExternalInput")
    WyF_T = nc.dram_tensor("WyF_T", [Cy, Cs], F32, kind="ExternalInput")
    WoF_T = nc.dram_tensor("WoF_T", [Cs, Cs], F32, kind="ExternalInput")
    Wy2F_T = nc.dram_tensor("Wy2F_T", [Cy, Cs], F32, kind="ExternalInput")
    Wq = nc.dram_tensor("Wq", [Cs, Cs], F32, kind="ExternalInput")
    Wk = nc.dram_tensor("Wk", [Cs, Cs], F32, kind="ExternalInput")
    Wv = nc.dram_tensor("Wv", [Cs, Cs], F32, kind="ExternalInput")
    w3T = nc.dram_tensor("w3T", [27, Cy, Cy], F32, kind="ExternalInput")
    BIAS = nc.dram_tensor("BIAS", [Cs, 4], F32, kind="ExternalInput")
    OUT = nc.dram_tensor("OUT", [repeat, 2 * Cs, KC], F32, kind="ExternalOutput")

    with tile.TileContext(nc) as tc:
        with (
            tc.tile_pool(name="const", bufs=1) as cp,
            tc.tile_pool(name="data", bufs=1) as dp,
            tc.tile_pool(name="chunk", bufs=3) as chp,
            tc.tile_pool(name="psum", bufs=2, space="PSUM") as pp,
            tc.tile_pool(name="dram", bufs=1, space="DRAM") as dram,
        ):
            # ---- constants into SBUF
            def load_const(t, shape):
                s = cp.tile(shape, F32, tag=t.name)
                nc.sync.dma_start(s[:], t.ap())
                return s

            wy = load_const(WyF_T, [Cy, Cs])
            ws = load_const(WsF_T, [Cs, Cs])
            wo = load_const(WoF_T, [Cs, Cs])
            wy2 = load_const(Wy2F_T, [Cy, Cs])
            wq = load_const(Wq, [Cs, Cs])
            wk = load_const(Wk, [Cs, Cs])
            wv = load_const(Wv, [Cs, Cs])
            bias = load_const(BIAS, [Cs, 4])
            w3 = cp.tile([Cy, 27, Cy], F32, tag="w3")
            nc.sync.dma_start(w3[:], w3T.ap().rearrange("t i o -> i t o"))

            for rep in range(repeat):
                sfx = f"r{rep}"
                # ---- per-core data
                yk = dp.tile([Cy, KC], F32, tag="yk")
                sk = dp.tile([Cs, KC], F32, tag="sk")
                yslab = dp.tile([Cy, 6, 18, 18], F32, tag="yslab")
                nc.sync.dma_start(yk[:], Yk.ap())
                nc.sync.dma_start(sk[:], Sk.ap())
                nc.sync.dma_start(yslab[:], Yslab.ap())

                q = dp.tile([Cs, N], F32, tag="q")
                k = dp.tile([Cs, KC], F32, tag="k")
                y1k = dp.tile([Cs, KC], F32, tag="y1k")
                s1k = dp.tile([Cs, KC], F32, tag="s1k")
                v = dp.tile([128, KT, Cs], F32, tag="v")
                vp = dp.tile([128, KT, Cs], BF16, tag="vp")
                e = dp.tile([128, KT, N], BF16, tag="e")
                z2 = dp.tile([128, KT, 2], F32, tag="z2")
                zr = dp.tile([128, KT], F32, tag="zr")

                # ---- Y1 chunks -> Q chunks (Y1 never fully materialized)
                for ci in range(N // QC):
                    yc = chp.tile([Cy, QC], F32, tag="yc")
                    nc.sync.dma_start(yc[:], Yf.ap()[:, ci * QC:(ci + 1) * QC])
                    ps1 = pp.tile([Cs, QC], F32, tag="ps")
                    nc.tensor.matmul(ps1[:], wy[:], yc[:], start=True, stop=True)
                    y1c = chp.tile([Cs, QC], F32, tag="y1c")
                    nc.scalar.activation(y1c[:], ps1[:], AF.Relu, bias=bias[:, 0:1])
                    ps2 = pp.tile([Cs, QC], F32, tag="ps")
                    nc.tensor.matmul(ps2[:], wq[:], y1c[:], start=True, stop=True)
                    nc.vector.tensor_copy(q[:, ci * QC:(ci + 1) * QC], ps2[:])

                # ---- Y1k -> K ; S1k -> V
                for ci in range(KC // QC):
                    sl = slice(ci * QC, (ci + 1) * QC)
                    ps1 = pp.tile([Cs, QC], F32, tag="ps")
                    nc.tensor.matmul(ps1[:], wy[:], yk[:, sl], start=True, stop=True)
                    nc.scalar.activation(y1k[:, sl], ps1[:], AF.Relu, bias=bias[:, 0:1])
                    ps2 = pp.tile([Cs, QC], F32, tag="ps")
                    nc.tensor.matmul(ps2[:], wk[:], y1k[:, sl], start=True, stop=True)
                    nc.vector.tensor_copy(k[:, sl], ps2[:])
                    ps3 = pp.tile([Cs, QC], F32, tag="ps")
                    nc.tensor.matmul(ps3[:], ws[:], sk[:, sl], start=True, stop=True)
                    nc.scalar.activation(s1k[:, sl], ps3[:], AF.Relu, bias=bias[:, 1:2])
                for kt in range(KT):
                    psv = pp.tile([128, Cs], F32, tag="ps")
                    nc.tensor.matmul(psv[:], s1k[:, kt * 128:(kt + 1) * 128], wv[:],
                                     start=True, stop=True)
                    nc.vector.tensor_copy(v[:, kt, :], psv[:])

                # ---- scores + exp + Z (per k-tile, 2 half-row PSUM tiles)
                for kt in range(KT):
                    lhs = k[:, kt * 128:(kt + 1) * 128]
                    for h in range(2):
                        pss = pp.tile([128, N // 2], F32, tag="ps")
                        for j in range(N // 2 // QC):
                            qs = slice(h * (N // 2) + j * QC,
                                       h * (N // 2) + (j + 1) * QC)
                            nc.tensor.matmul(pss[:, j * QC:(j + 1) * QC],
                                             lhs, q[:, qs], start=True, stop=True)
                        nc.scalar.activation(
                            e[:, kt, h * (N // 2):(h + 1) * (N // 2)], pss[:],
                            AF.Exp, accum_out=z2[:, kt, h:h + 1])

                # ---- V' = V / Z (bf16)
                nc.vector.tensor_add(zr[:, :], z2[:, :, 0], z2[:, :, 1])
                nc.vector.reciprocal(zr[:, :], zr[:, :])
                for kt in range(KT):
                    nc.vector.tensor_scalar_mul(vp[:, kt, :], v[:, kt, :],
                                                zr[:, kt:kt + 1])

                # ---- x^T partials -> DRAM collective input
                cin = dram.tile([G * Cs, KC], F32, tag="cin")
                cout = dram.tile([Cs, KC], F32, tag="cout")
                for ci in range(N // QC):
                    psx = pp.tile([Cs, QC], F32, tag="ps")
                    for kt in range(KT):
                        nc.tensor.matmul(psx[:], vp[:, kt, :],
                                         e[:, kt, ci * QC:(ci + 1) * QC],
                                         start=(kt == 0), stop=(kt == KT - 1))
                    xc = chp.tile([Cs, QC], F32, tag="xc")
                    nc.vector.tensor_copy(xc[:], psx[:])
                    gg, half = divmod(ci, 2)
                    nc.sync.dma_start(
                        cin[gg * Cs:(gg + 1) * Cs, half * QC:(half + 1) * QC], xc[:])

                nc.gpsimd.collective_compute(
                    "ReduceScatter", mybir.AluOpType.add,
                    replica_groups=[[0, 1, 2, 3], [4, 5, 6, 7]],
                    ins=[cin[:]], outs=[cout[:]],
                )

                # ---- conv3x3x3 (no dep on RS -> fills the RS wait) + Y2
                c3 = dp.tile([Cy, KC], F32, tag="c3")
                y2 = dp.tile([Cs, KC], F32, tag="y2")
                for ci in range(2):
                    psc = pp.tile([Cy, QC], F32, tag="ps")
                    for t in range(27):
                        kd, r = divmod(t, 9)
                        kh, kw = divmod(r, 3)
                        nc.tensor.matmul(
                            psc[:],
                            w3[:, t, :],
                            yslab[:, 2 * ci + kd:2 * ci + kd + 2,
                                  kh:kh + 16, kw:kw + 16],
                            start=(t == 0), stop=(t == 26))
                    nc.vector.tensor_copy(c3[:, ci * QC:(ci + 1) * QC], psc[:])
                for ci in range(2):
                    sl = slice(ci * QC, (ci + 1) * QC)
                    psy = pp.tile([Cs, QC], F32, tag="ps")
                    nc.tensor.matmul(psy[:], wy2[:], c3[:, sl], start=True, stop=True)
                    nc.scalar.activation(y2[:, sl], psy[:], AF.Relu, bias=bias[:, 3:4])
                nc.sync.dma_start(OUT.ap()[rep, Cs:2 * Cs, :], y2[:])

                # ---- post-RS: out-projection, mul by S+pe
                xr = dp.tile([Cs, KC], F32, tag="xr")
                nc.sync.dma_start(xr[:], cout[:])
                zc = dp.tile([Cs, KC], F32, tag="zc")
                zo = dp.tile([Cs, KC], F32, tag="zo")
                for ci in range(2):
                    sl = slice(ci * QC, (ci + 1) * QC)
                    psz = pp.tile([Cs, QC], F32, tag="ps")
                    nc.tensor.matmul(psz[:], wo[:], xr[:, sl], start=True, stop=True)
                    nc.scalar.activation(zc[:, sl], psz[:], AF.Relu, bias=bias[:, 2:3])
                nc.vector.tensor_mul(zo[:], zc[:], sk[:])
                nc.sync.dma_start(OUT.ap()[rep, 0:Cs, :], zo[:])

    nc.compile()
    return nc


def _get(repeat=1):
    if repeat not in _cache:
        _cache[repeat] = _build(repeat)
    return _cache[repeat]


# ---------------------------------------------------------------- entry point
def kernel(**inputs):
    in_maps = _prepare(inputs)
    nc = _get(1)
    res = run_bass_kernel_spmd(nc, in_maps, core_ids=list(range(NC)), trace=False)
    out = np.zeros((B, 2 * Cs, D, H, W), np.float32)
    for core in range(NC):
        b, g = divmod(core, G)
        blk = res.results[core]["OUT"][0].reshape(2 * Cs, D // G, H, W)
        out[b, :, g * (D // G):(g + 1) * (D // G)] = blk
    return out


# revision 5
# speedup vs baseline: 1.8734x; 1.8734x over previous
"""Trainium2 Bass kernel for nn_MultiHeadCrossAttention_57638461112647.

Sharding: 8 cores = 2 batches x 4-way split over attention *keys* (and,
identically, over output tokens). The softmax in the reference is over the
*query* axis (axis=1), so with scores laid out (keys on partitions, queries on
free) the softmax denominator Z[k] is a free-axis row-sum, fully core-local.
The only cross-core exchange is one ReduceScatter of the attention output
partials x^T = V'^T E (48 x 4096) within each 4-core batch group.

Host-side prep (cheap, O(C^2) / O(N*C)): fold BN-as-affine + biases into the
1x1-conv weights, fold the attention scale into Wq, fold the 3x3x3-conv bias
through the following 1x1 conv, add the (constant) 3D positional encodings,
pad the conv input with its halo, and slice per-core chunks.

Device pipeline per core (channel-major layouts, channels on partitions):
  Y1 = relu(WyF @ (Y+pe))          -> Q^T = Wq'^T Y1 (48x4096, all queries)
  Y1k/S1k from pre-sliced chunks   -> K^T (48x1024), V (1024x48, own keys)
  scores^T = K^T^T Q^T             (PE, 8 k-tiles of 128 x 4096 q)
  E = exp(scores), Z = rowsum      (ACT, accum_out fused; no max-sub needed:
                                    scores are O(0.2))
  V' = V / Z (bf16), E bf16        -> x^T partial = sum_kt V'[kt]^T E[kt] (PE)
  ReduceScatter(4-core group)      -> x^T reduced for own 1024 tokens
  out  rows 0-47: relu(WoF @ x^T + b) * (S+pe)[own chunk]
  conv3x3x3 as 27 accumulating matmuls on a padded slab (runs during the RS)
  out rows 48-95: relu(Wy2F @ conv3 + b')
"""
import numpy as np
import ml_dtypes
import jax
from jax.sharding import Mesh, PartitionSpec
from jax.experimental.shard_map import shard_map

import concourse.bass as bass
import concourse.mybir as mybir
import concourse.tile as tile
from concourse import bacc
from concourse import bass2jax
from concourse.bass2jax import _bass_exec_p, install_neuronx_cc_hook

F32 = mybir.dt.float32
BF16 = mybir.dt.bfloat16
AF = mybir.ActivationFunctionType

B, Cy, Cs, D, H, W = 2, 96, 48, 16, 16, 16
N = D * H * W            # 4096 tokens
NC = 8                   # cores
G = 4                    # cores per batch
KC = N // G              # keys / output tokens per core = 1024
KT = KC // 128           # k-tiles per core = 8
QC = 512                 # q (free-dim) chunk per matmul
EPS = 1e-5

_cache = {}


# ---------------------------------------------------------------- host prep
def _pe3d(C, x, y, z):
    """Transcription of reference.pe3d (incl. its quirky torch broadcasting)."""
    c = int(np.ceil(C / 3))
    inv_freq = (1.0 / (10000.0 ** (np.arange(0, c, 2, dtype=np.float32) / c))
                ).astype(np.float32)

    def emb(n):
        s = np.arange(n, dtype=np.float32)[:, None] * inv_freq[None, :]
        return np.concatenate([np.sin(s), np.cos(s)], axis=-1).astype(np.float32)

    out = np.zeros((x, y, z, 3 * c), np.float32)
    out[..., :c] = emb(x)[:, None, :]        # broadcasts against (y, z, c)
    out[..., c:2 * c] = emb(y)[:, None, :]
    out[..., 2 * c:3 * c] = emb(z)
    return np.ascontiguousarray(out[..., :C].transpose(3, 0, 1, 2))  # (C,x,y,z)


def _prepare(inputs):
    f = lambda a: np.ascontiguousarray(np.asarray(a, np.float32))
    Y, S = f(inputs['Y']), f(inputs['S'])

    pe_s = _pe3d(Cs, D, H, W).reshape(Cs, N)
    pe_y = _pe3d(Cy, D, H, W).reshape(Cy, N)
    Scm = S.reshape(B, Cs, N) + pe_s[None]
    Ycm = Y.reshape(B, Cy, N) + pe_y[None]

    sb = lambda g: f(g) / np.sqrt(np.float32(1.0) + np.float32(EPS))

    def fold(w, b, g, be):
        s = sb(g)
        return f(w) * s[:, None], (f(b) * s + f(be)).astype(np.float32)

    WsF, bsF = fold(inputs['w_s'], inputs['b_s'], inputs['g_s'], inputs['be_s'])
    WyF, byF = fold(inputs['w_y'], inputs['b_y'], inputs['g_y'], inputs['be_y'])
    WoF, boF = fold(inputs['w_o'], inputs['b_o'], inputs['g_o'], inputs['be_o'])
    Wy2F, by2F = fold(inputs['w_y2'], inputs['b_y2'], inputs['g_y2'], inputs['be_y2'])
    by2FF = (Wy2F @ f(inputs['b3']) + by2F).astype(np.float32)

    c = np.ascontiguousarray
    Wq = c(f(inputs['Wq']) * np.float32(Cs) ** np.float32(-0.5))
    Wk = c(f(inputs['Wk']))
    Wv = c(f(inputs['Wv']))
    WsF_T, WyF_T = c(WsF.T), c(WyF.T)
    WoF_T, Wy2F_T = c(WoF.T), c(Wy2F.T)
    w3T = c(f(inputs['w3']).reshape(Cy, Cy, 27).transpose(2, 1, 0))  # (27,96,96)
    BIAS = c(np.stack([byF, bsF, boF, by2FF], axis=1))               # (48,4)

    Ypad = np.zeros((B, Cy, D + 2, H + 2, W + 2), np.float32)
    Ypad[:, :, 1:-1, 1:-1, 1:-1] = Ycm.reshape(B, Cy, D, H, W)

    in_maps = []
    for core in range(NC):
        b, g = divmod(core, G)
        d0 = g * (D // G)
        in_maps.append(dict(
            Yf=c(Ycm[b]),                                  # (96,4096)
            Yk=c(Ycm[b, :, g * KC:(g + 1) * KC]),          # (96,1024)
            Sk=c(Scm[b, :, g * KC:(g + 1) * KC]),          # (48,1024)
            Yslab=c(Ypad[b, :, d0:d0 + 6, :, :]),          # (96,6,18,18)
            WsF_T=WsF_T, WyF_T=WyF_T, WoF_T=WoF_T, Wy2F_T=Wy2F_T,
            Wq=Wq, Wk=Wk, Wv=Wv, w3T=w3T, BIAS=BIAS,
        ))
    return in_maps


# ---------------------------------------------------------------- bass build
def _build(repeat=1):
    nc = bacc.Bacc("TRN2", target_bir_lowering=False, debug=False, num_devices=NC)

    Yf = nc.dram_tensor("Yf", [Cy, N], F32, kind="ExternalInput")
    Yk = nc.dram_tensor("Yk", [Cy, KC], F32, kind="ExternalInput")
    Sk = nc.dram_tensor("Sk", [Cs, KC], F32, kind="ExternalInput")
    Yslab = nc.dram_tensor("Yslab", [Cy, 6, 18, 18], F32, kind="ExternalInput")
    WsF_T = nc.dram_tensor("WsF_T", [Cs, Cs], F32, kind="ExternalInput")
    WyF_T = nc.dram_tensor("WyF_T", [Cy, Cs], F32, kind="ExternalInput")
    WoF_T = nc.dram_tensor("WoF_T", [Cs, Cs], F32, kind="ExternalInput")
    Wy2F_T = nc.dram_tensor("Wy2F_T", [Cy, Cs], F32, kind="ExternalInput")
    Wq = nc.dram_tensor("Wq", [Cs, Cs], F32, kind="ExternalInput")
    Wk = nc.dram_tensor("Wk", [Cs, Cs], F32, kind="ExternalInput")
    Wv = nc.dram_tensor("Wv", [Cs, Cs], F32, kind="ExternalInput")
    w3T = nc.dram_tensor("w3T", [27, Cy, Cy], F32, kind="ExternalInput")
    BIAS = nc.dram_tensor("BIAS", [Cs, 4], F32, kind="ExternalInput")
    OUT = nc.dram_tensor("OUT", [repeat, 2 * Cs, KC], F32, kind="ExternalOutput")

    with tile.TileContext(nc) as tc:
        with (
            tc.tile_pool(name="const", bufs=1) as cp,
            tc.tile_pool(name="data", bufs=1) as dp,
            tc.tile_pool(name="chunk", bufs=3) as chp,
            tc.tile_pool(name="psum", bufs=2, space="PSUM") as pp,
            tc.tile_pool(name="dram", bufs=1, space="DRAM") as dram,
        ):
            # ---- constants into SBUF
            def load_const(t, shape):
                s = cp.tile(shape, F32, tag=t.name)
                nc.sync.dma_start(s[:], t.ap())
                return s

            wy = load_const(WyF_T, [Cy, Cs])
            ws = load_const(WsF_T, [Cs, Cs])
            wo = load_const(WoF_T, [Cs, Cs])
            wy2 = load_const(Wy2F_T, [Cy, Cs])
            wq = load_const(Wq, [Cs, Cs])
            wk = load_const(Wk, [Cs, Cs])
            wv = load_const(Wv, [Cs, Cs])
            bias = load_const(BIAS, [Cs, 4])
            w3 = cp.tile([Cy, 27, Cy], F32, tag="w3")
            nc.sync.dma_start(w3[:], w3T.ap().rearrange("t i o -> i t o"))

            for rep in range(repeat):
                sfx = f"r{rep}"
                # ---- per-core data
                yk = dp.tile([Cy, KC], F32, tag="yk")
                sk = dp.tile([Cs, KC], F32, tag="sk")
                yslab = dp.tile([Cy, 6, 18, 18], F32, tag="yslab")
                nc.sync.dma_start(yk[:], Yk.ap())
                nc.sync.dma_start(sk[:], Sk.ap())
                nc.sync.dma_start(yslab[:], Yslab.ap())

                q = dp.tile([Cs, N], F32, tag="q")
                k = dp.tile([Cs, KC], F32, tag="k")
                y1k = dp.tile([Cs, KC], F32, tag="y1k")
                s1k = dp.tile([Cs, KC], F32, tag="s1k")
                v = dp.tile([128, KT, Cs], F32, tag="v")
                vp = dp.tile([128, KT, Cs], BF16, tag="vp")
                e = dp.tile([128, KT, N], BF16, tag="e")
                z2 = dp.tile([128, KT, 2], F32, tag="z2")
                zr = dp.tile([128, KT], F32, tag="zr")

                # ---- Y1 chunks -> Q chunks (Y1 never fully materialized)
                for ci in range(N // QC):
                    yc = chp.tile([Cy, QC], F32, tag="yc")
                    nc.sync.dma_start(yc[:], Yf.ap()[:, ci * QC:(ci + 1) * QC])
                    ps1 = pp.tile([Cs, QC], F32, tag="ps")
                    nc.tensor.matmul(ps1[:], wy[:], yc[:], start=True, stop=True)
                    y1c = chp.tile([Cs, QC], F32, tag="y1c")
                    nc.scalar.activation(y1c[:], ps1[:], AF.Relu, bias=bias[:, 0:1])
                    ps2 = pp.tile([Cs, QC], F32, tag="ps")
                    nc.tensor.matmul(ps2[:], wq[:], y1c[:], start=True, stop=True)
                    nc.vector.tensor_copy(q[:, ci * QC:(ci + 1) * QC], ps2[:])

                # ---- Y1k -> K ; S1k -> V
                for ci in range(KC // QC):
                    sl = slice(ci * QC, (ci + 1) * QC)
                    ps1 = pp.tile([Cs, QC], F32, tag="ps")
                    nc.tensor.matmul(ps1[:], wy[:], yk[:, sl], start=True, stop=True)
                    nc.scalar.activation(y1k[:, sl], ps1[:], AF.Relu, bias=bias[:, 0:1])
                    ps2 = pp.tile([Cs, QC], F32, tag="ps")
                    nc.tensor.matmul(ps2[:], wk[:], y1k[:, sl], start=True, stop=True)
                    nc.vector.tensor_copy(k[:, sl], ps2[:])
                    ps3 = pp.tile([Cs, QC], F32, tag="ps")
                    nc.tensor.matmul(ps3[:], ws[:], sk[:, sl], start=True, stop=True)
                    nc.scalar.activation(s1k[:, sl], ps3[:], AF.Relu, bias=bias[:, 1:2])
                for kt in range(KT):
                    psv = pp.tile([128, Cs], F32, tag="ps")
                    nc.tensor.matmul(psv[:], s1k[:, kt * 128:(kt + 1) * 128], wv[:],
                                     start=True, stop=True)
                    nc.vector.tensor_copy(v[:, kt, :], psv[:])

                # ---- scores + exp + Z (per k-tile, 2 half-row PSUM tiles)
                for kt in range(KT):
                    lhs = k[:, kt * 128:(kt + 1) * 128]
                    for h in range(2):
                        pss = pp.tile([128, N // 2], F32, tag="ps")
                        for j in range(N // 2 // QC):
                            qs = slice(h * (N // 2) + j * QC,
                                       h * (N // 2) + (j + 1) * QC)
                            nc.tensor.matmul(pss[:, j * QC:(j + 1) * QC],
                                             lhs, q[:, qs], start=True, stop=True)
                        nc.scalar.activation(
                            e[:, kt, h * (N // 2):(h + 1) * (N // 2)], pss[:],
                            AF.Exp, accum_out=z2[:, kt, h:h + 1])

                # ---- V' = V / Z (bf16)
                nc.vector.tensor_add(zr[:, :], z2[:, :, 0], z2[:, :, 1])
                nc.vector.reciprocal(zr[:, :], zr[:, :])
                for kt in range(KT):
                    nc.vector.tensor_scalar_mul(vp[:, kt, :], v[:, kt, :],
                                                zr[:, kt:kt + 1])

                # ---- x^T partials -> DRAM collective input
                cin = dram.tile([G * Cs, KC], F32, tag="cin")
                cout = dram.tile([Cs, KC], F32, tag="cout")
                for ci in range(N // QC):
                    psx = pp.tile([Cs, QC], F32, tag="ps")
                    for kt in range(KT):
                        nc.tensor.matmul(psx[:], vp[:, kt, :],
                                         e[:, kt, ci * QC:(ci + 1) * QC],
                                         start=(kt == 0), stop=(kt == KT - 1))
                    xc = chp.tile([Cs, QC], F32, tag="xc")
                    nc.vector.tensor_copy(xc[:], psx[:])
                    gg, half = divmod(ci, 2)
                    nc.sync.dma_start(
                        cin[gg * Cs:(gg + 1) * Cs, half * QC:(half + 1) * QC], xc[:])

                nc.gpsimd.collective_compute(
                    "ReduceScatter", mybir.AluOpType.add,
                    replica_groups=[[0, 1, 2, 3], [4, 5, 6, 7]],
                    ins=[cin[:]], outs=[cout[:]],
                )

                # ---- conv3x3x3 (no dep on RS -> fills the RS wait) + Y2
                c3 = dp.tile([Cy, KC], F32, tag="c3")
                y2 = dp.tile([Cs, KC], F32, tag="y2")
                for ci in range(2):
                    psc = pp.tile([Cy, QC], F32, tag="ps")
                    for t in range(27):
                        kd, r = divmod(t, 9)
                        kh, kw = divmod(r, 3)
                        nc.tensor.matmul(
                            psc[:],
                            w3[:, t, :],
                            yslab[:, 2 * ci + kd:2 * ci + kd + 2,
                                  kh:kh + 16, kw:kw + 16],
                            start=(t == 0), stop=(t == 26))
                    nc.vector.tensor_copy(c3[:, ci * QC:(ci + 1) * QC], psc[:])
                for ci in range(2):
                    sl = slice(ci * QC, (ci + 1) * QC)
                    psy = pp.tile([Cs, QC], F32, tag="ps")
                    nc.tensor.matmul(psy[:], wy2[:], c3[:, sl], start=True, stop=True)
                    nc.scalar.activation(y2[:, sl], psy[:], AF.Relu, bias=bias[:, 3:4])
                nc.sync.dma_start(OUT.ap()[rep, Cs:2 * Cs, :], y2[:])

                # ---- post-RS: out-projection, mul by S+pe
                xr = dp.tile([Cs, KC], F32, tag="xr")
                nc.sync.dma_start(xr[:], cout[:])
                zc = dp.tile([Cs, KC], F32, tag="zc")
                zo = dp.tile([Cs, KC], F32, tag="zo")
                for ci in range(2):
                    sl = slice(ci * QC, (ci + 1) * QC)
                    psz = pp.tile([Cs, QC], F32, tag="ps")
                    nc.tensor.matmul(psz[:], wo[:], xr[:, sl], start=True, stop=True)
                    nc.scalar.activation(zc[:, sl], psz[:], AF.Relu, bias=bias[:, 2:3])
                nc.vector.tensor_mul(zo[:], zc[:], sk[:])
                nc.sync.dma_start(OUT.ap()[rep, 0:Cs, :], zo[:])

    nc.compile()
    return nc


class _Runner:
    """Builds the bass module once and a single reusable jitted callable
    (re-jitting per call would re-trace + re-hash the BIR module: ~600ms)."""

    def __init__(self, repeat=1):
        install_neuronx_cc_hook()
        nc = _build(repeat)
        pid = nc.partition_id_tensor.name if nc.partition_id_tensor else None
        in_names, out_names, out_avals = [], [], []
        for alloc in nc.m.functions[0].allocations:
            if not isinstance(alloc, mybir.MemoryLocationSet):
                continue
            name = alloc.memorylocations[0].name
            if alloc.kind == "ExternalInput":
                if name != pid:
                    in_names.append(name)
            elif alloc.kind == "ExternalOutput":
                out_names.append(name)
                out_avals.append(jax.core.ShapedArray(
                    tuple(alloc.tensor_shape), mybir.dt.np(alloc.dtype)))
        self.in_names, self.out_names, self.out_avals = in_names, out_names, out_avals
        all_names = in_names + out_names + ([pid] if pid else [])

        def _body(*args):
            operands = list(args)
            if pid is not None:
                operands.append(bass2jax.partition_id_tensor())
            return tuple(_bass_exec_p.bind(
                *operands, out_avals=tuple(out_avals), in_names=tuple(all_names),
                out_names=tuple(out_names), lowering_input_output_aliases=(),
                sim_require_finite=True, sim_require_nnan=True, nc=nc))

        mesh = Mesh(np.asarray(jax.devices()[:NC]), ("core",))
        sp = (PartitionSpec("core"),)
        n_in = len(in_names) + len(out_names)
        self.fn = jax.jit(
            shard_map(_body, mesh=mesh, in_specs=sp * n_in,
                      out_specs=sp * len(out_names), check_rep=False),
            keep_unused=True)

    def __call__(self, in_maps):
        cat = [np.concatenate([in_maps[c][n] for c in range(NC)], axis=0)
               for n in self.in_names]
        zz = [np.zeros((NC * a.shape[0], *a.shape[1:]), a.dtype)
              for a in self.out_avals]
        outs = self.fn(*cat, *zz)
        jax.block_until_ready(outs)
        return [
            {n: np.asarray(outs[i]).reshape(NC, *self.out_avals[i].shape)[c]
             for i, n in enumerate(self.out_names)}
            for c in range(NC)
        ]


def _get(repeat=1):
    if repeat not in _cache:
        _cache[repeat] = _Runner(repeat)
    return _cache[repeat]


# ---------------------------------------------------------------- entry point
def kernel(**inputs):
    in_maps = _prepare(inputs)
    results = _get(1)(in_maps)
    out = np.zeros((B, 2 * Cs, D, H, W), np.float32)
    for core in range(NC):
        b, g = divmod(core, G)
        blk = results[core]["OUT"][0].reshape(2 * Cs, D // G, H, W)
        out[b, :, g * (D // G):(g + 1) * (D // G)] = blk
    return out


# revision 19
# speedup vs baseline: 98.0729x; 52.3512x over previous
"""Trainium2 Bass kernel for nn_MultiHeadCrossAttention_57638461112647.

Sharding: 8 cores = 2 batches x 4-way split over attention *keys* (and,
identically, over output tokens). The softmax in the reference is over the
*query* axis (axis=1), so with scores laid out (keys on partitions, queries on
free) the softmax denominator Z[k] is a free-axis row-sum, fully core-local.
The only cross-core exchange is one ReduceScatter of the attention output
partials x^T = V'^T E (48 x 4096) within each 4-core batch group.

Host-side prep (cheap, O(C^2) / O(N*C)): fold BN-as-affine + biases into the
1x1-conv weights, fold the attention scale into Wq, fold the 3x3x3-conv bias
through the following 1x1 conv, add the (constant) 3D positional encodings,
pad the conv input with its halo, and slice per-core chunks.

Device pipeline per core (channel-major layouts, channels on partitions):
  Y1 = relu(WyF @ (Y+pe))          -> Q^T = Wq'^T Y1 (48x4096, all queries)
  Y1k/S1k from pre-sliced chunks   -> K^T (48x1024), V (1024x48, own keys)
  scores^T = K^T^T Q^T             (PE, 8 k-tiles of 128 x 4096 q)
  E = exp(scores), Z = rowsum      (ACT, accum_out fused; no max-sub needed:
                                    scores are O(0.2))
  V' = V / Z (bf16), E bf16        -> x^T partial = sum_kt V'[kt]^T E[kt] (PE)
  ReduceScatter(4-core group)      -> x^T reduced for own 1024 tokens
  out  rows 0-47: relu(WoF @ x^T + b) * (S+pe)[own chunk]
  conv3x3x3 as 27 accumulating matmuls on a padded slab (runs during the RS)
  out rows 48-95: relu(Wy2F @ conv3 + b')
"""
import numpy as np
import ml_dtypes
import jax
from jax.sharding import Mesh, PartitionSpec
from jax.experimental.shard_map import shard_map

import concourse.bass as bass
import concourse.mybir as mybir
import concourse.tile as tile
from concourse import bacc
from concourse import bass2jax
from concourse.bass2jax import _bass_exec_p, install_neuronx_cc_hook

F32 = mybir.dt.float32
BF16 = mybir.dt.bfloat16
AF = mybir.ActivationFunctionType

B, Cy, Cs, D, H, W = 2, 96, 48, 16, 16, 16
N = D * H * W            # 4096 tokens
NC = 8                   # cores
G = 4                    # cores per batch
KC = N // G              # keys / output tokens per core = 1024
KT = KC // 128           # k-tiles per core = 8
QC = 512                 # q (free-dim) chunk per matmul
EPS = 1e-5

_cache = {}


# ---------------------------------------------------------------- host prep
def _pe3d(C, x, y, z):
    """Transcription of reference.pe3d (incl. its quirky torch broadcasting)."""
    c = int(np.ceil(C / 3))
    inv_freq = (1.0 / (10000.0 ** (np.arange(0, c, 2, dtype=np.float32) / c))
                ).astype(np.float32)

    def emb(n):
        s = np.arange(n, dtype=np.float32)[:, None] * inv_freq[None, :]
        return np.concatenate([np.sin(s), np.cos(s)], axis=-1).astype(np.float32)

    out = np.zeros((x, y, z, 3 * c), np.float32)
    out[..., :c] = emb(x)[:, None, :]        # broadcasts against (y, z, c)
    out[..., c:2 * c] = emb(y)[:, None, :]
    out[..., 2 * c:3 * c] = emb(z)
    return np.ascontiguousarray(out[..., :C].transpose(3, 0, 1, 2))  # (C,x,y,z)


def _prepare(inputs):
    f = lambda a: np.ascontiguousarray(np.asarray(a, np.float32))
    Y, S = f(inputs['Y']), f(inputs['S'])

    pe_s = _pe3d(Cs, D, H, W).reshape(Cs, N)
    pe_y = _pe3d(Cy, D, H, W).reshape(Cy, N)
    Scm = S.reshape(B, Cs, N) + pe_s[None]
    Ycm = Y.reshape(B, Cy, N) + pe_y[None]

    sb = lambda g: f(g) / np.sqrt(np.float32(1.0) + np.float32(EPS))

    def fold(w, b, g, be):
        s = sb(g)
        return f(w) * s[:, None], (f(b) * s + f(be)).astype(np.float32)

    WsF, bsF = fold(inputs['w_s'], inputs['b_s'], inputs['g_s'], inputs['be_s'])
    WyF, byF = fold(inputs['w_y'], inputs['b_y'], inputs['g_y'], inputs['be_y'])
    WoF, boF = fold(inputs['w_o'], inputs['b_o'], inputs['g_o'], inputs['be_o'])
    Wy2F, by2F = fold(inputs['w_y2'], inputs['b_y2'], inputs['g_y2'], inputs['be_y2'])
    by2FF = (Wy2F @ f(inputs['b3']) + by2F).astype(np.float32)

    c = np.ascontiguousarray
    Wq = c(f(inputs['Wq']) * np.float32(Cs) ** np.float32(-0.5))
    Wk = c(f(inputs['Wk']))
    Wv = c(f(inputs['Wv']))
    WsF_T, WyF_T = c(WsF.T), c(WyF.T)
    WoF_T, Wy2F_T = c(WoF.T), c(Wy2F.T)
    w3T = c(f(inputs['w3']).reshape(Cy, Cy, 27).transpose(2, 1, 0))  # (27,96,96)
    BIAS = c(np.stack([byF, bsF, boF, by2FF], axis=1))               # (48,4)

    Ypad = np.zeros((B, Cy, D + 2, H + 2, W + 2), np.float32)
    Ypad[:, :, 1:-1, 1:-1, 1:-1] = Ycm.reshape(B, Cy, D, H, W)

    in_maps = []
    for core in range(NC):
        b, g = divmod(core, G)
        d0 = g * (D // G)
        in_maps.append(dict(
            Yf=c(Ycm[b]),                                  # (96,4096)
            Yk=c(Ycm[b, :, g * KC:(g + 1) * KC]),          # (96,1024)
            Sk=c(Scm[b, :, g * KC:(g + 1) * KC]),          # (48,1024)
            Yslab=c(Ypad[b, :, d0:d0 + 6, :, :]),          # (96,6,18,18)
            WsF_T=WsF_T, WyF_T=WyF_T, WoF_T=WoF_T, Wy2F_T=Wy2F_T,
            Wq=Wq, Wk=Wk, Wv=Wv, w3T=w3T, BIAS=BIAS,
        ))
    return in_maps


# ---------------------------------------------------------------- bass build
def _build(repeat=1, ablate=(), score_fd=2048, exp_accum=True, rs_bf16=True):
    """ablate: subset of {'rs','attn','conv','proj','qkv'} — for timing bisection
    only (results become wrong)."""
    nc = bacc.Bacc("TRN2", target_bir_lowering=False, debug=False, num_devices=NC)

    Yf = nc.dram_tensor("Yf", [Cy, N], F32, kind="ExternalInput")
    Yk = nc.dram_tensor("Yk", [Cy, KC], F32, kind="ExternalInput")
    Sk = nc.dram_tensor("Sk", [Cs, KC], F32, kind="ExternalInput")
    Yslab = nc.dram_tensor("Yslab", [Cy, 6, 18, 18], F32, kind="ExternalInput")
    WsF_T = nc.dram_tensor("WsF_T", [Cs, Cs], F32, kind="ExternalInput")
    WyF_T = nc.dram_tensor("WyF_T", [Cy, Cs], F32, kind="ExternalInput")
    WoF_T = nc.dram_tensor("WoF_T", [Cs, Cs], F32, kind="ExternalInput")
    Wy2F_T = nc.dram_tensor("Wy2F_T", [Cy, Cs], F32, kind="ExternalInput")
    Wq = nc.dram_tensor("Wq", [Cs, Cs], F32, kind="ExternalInput")
    Wk = nc.dram_tensor("Wk", [Cs, Cs], F32, kind="ExternalInput")
    Wv = nc.dram_tensor("Wv", [Cs, Cs], F32, kind="ExternalInput")
    w3T = nc.dram_tensor("w3T", [27, Cy, Cy], F32, kind="ExternalInput")
    BIAS = nc.dram_tensor("BIAS", [Cs, 4], F32, kind="ExternalInput")
    OUT = nc.dram_tensor("OUT", [2 * Cs, KC], F32, kind="ExternalOutput")

    with tile.TileContext(nc) as tc:
        with (
            tc.tile_pool(name="const", bufs=1) as cp,
            tc.tile_pool(name="data", bufs=1) as dp,
            tc.tile_pool(name="chunk", bufs=3) as chp,
            tc.tile_pool(name="psum", bufs=2, space="PSUM") as pp,
            tc.tile_pool(name="dram", bufs=1, space="DRAM") as dram,
        ):
            # ---- constants into SBUF
            def load_const(t, shape):
                s = cp.tile(shape, F32, tag=t.name)
                nc.sync.dma_start(s[:], t.ap())
                return s

            wy = load_const(WyF_T, [Cy, Cs])
            ws = load_const(WsF_T, [Cs, Cs])
            wo = load_const(WoF_T, [Cs, Cs])
            wy2 = load_const(Wy2F_T, [Cy, Cs])
            wq = load_const(Wq, [Cs, Cs])
            wk = load_const(Wk, [Cs, Cs])
            wv = load_const(Wv, [Cs, Cs])
            bias = load_const(BIAS, [Cs, 4])
            w3 = cp.tile([Cy, 27, Cy], F32, tag="w3")
            nc.sync.dma_start(w3[:], w3T.ap().rearrange("t i o -> i t o"))

            for rep in range(repeat):
                sfx = f"r{rep}"
                # ---- per-core data
                yk = dp.tile([Cy, KC], F32, tag="yk")
                sk = dp.tile([Cs, KC], F32, tag="sk")
                yslab = dp.tile([Cy, 6, 18, 18], F32, tag="yslab")
                nc.sync.dma_start(yk[:], Yk.ap())
                nc.sync.dma_start(sk[:], Sk.ap())
                nc.sync.dma_start(yslab[:], Yslab.ap())

                q = dp.tile([Cs, N], F32, tag="q")
                k = dp.tile([Cs, KC], F32, tag="k")
                y1k = dp.tile([Cs, KC], F32, tag="y1k")
                s1k = dp.tile([Cs, KC], F32, tag="s1k")
                v = dp.tile([128, KT, Cs], F32, tag="v")
                vp = dp.tile([128, KT, Cs], BF16, tag="vp")
                e = dp.tile([128, KT, N], BF16, tag="e")
                z2 = dp.tile([128, KT, 4], F32, tag="z2")
                zr = dp.tile([128, KT], F32, tag="zr")

                # ---- Y1 chunks -> Q chunks (Y1 never fully materialized)
                for ci in range(N // QC if 'qkv' not in ablate else 0):
                    yc = chp.tile([Cy, QC], F32, tag="yc")
                    nc.sync.dma_start(yc[:], Yf.ap()[:, ci * QC:(ci + 1) * QC])
                    ps1 = pp.tile([Cs, QC], F32, tag="ps")
                    nc.tensor.matmul(ps1[:], wy[:], yc[:], start=True, stop=True)
                    y1c = chp.tile([Cs, QC], F32, tag="y1c")
                    nc.vector.tensor_scalar(y1c[:], ps1[:], bias[:, 0:1], 0.0,
                                            mybir.AluOpType.add, mybir.AluOpType.max)
                    ps2 = pp.tile([Cs, QC], F32, tag="ps")
                    nc.tensor.matmul(ps2[:], wq[:], y1c[:], start=True, stop=True)
                    nc.vector.tensor_copy(q[:, ci * QC:(ci + 1) * QC], ps2[:])

                # ---- Y1k -> K ; S1k -> V
                for ci in range(KC // QC if 'qkv' not in ablate else 0):
                    sl = slice(ci * QC, (ci + 1) * QC)
                    ps1 = pp.tile([Cs, QC], F32, tag="ps")
                    nc.tensor.matmul(ps1[:], wy[:], yk[:, sl], start=True, stop=True)
                    nc.vector.tensor_scalar(y1k[:, sl], ps1[:], bias[:, 0:1], 0.0,
                                            mybir.AluOpType.add, mybir.AluOpType.max)
                    ps2 = pp.tile([Cs, QC], F32, tag="ps")
                    nc.tensor.matmul(ps2[:], wk[:], y1k[:, sl], start=True, stop=True)
                    nc.vector.tensor_copy(k[:, sl], ps2[:])
                    ps3 = pp.tile([Cs, QC], F32, tag="ps")
                    nc.tensor.matmul(ps3[:], ws[:], sk[:, sl], start=True, stop=True)
                    nc.vector.tensor_scalar(s1k[:, sl], ps3[:], bias[:, 1:2], 0.0,
                                            mybir.AluOpType.add, mybir.AluOpType.max)
                for kt in range(KT if 'qkv' not in ablate else 0):
                    psv = pp.tile([128, Cs], F32, tag="ps")
                    nc.tensor.matmul(psv[:], s1k[:, kt * 128:(kt + 1) * 128], wv[:],
                                     start=True, stop=True)
                    nc.vector.tensor_copy(v[:, kt, :], psv[:])

                # ---- scores + exp + Z (per k-tile, PSUM tiles of score_fd)
                if 'attn' in ablate or 'qkv' in ablate:
                    nc.gpsimd.memset(z2[:], 1.0)
                    nc.gpsimd.memset(v[:], 0.5)
                    nc.gpsimd.memset(e[:], 0.25)
                n_h = N // score_fd
                assert z2.shape[2] >= n_h or not exp_accum
                for kt in range(KT if 'attn' not in ablate and 'qkv' not in ablate
                                else 0):
                    lhs = k[:, kt * 128:(kt + 1) * 128]
                    for h in range(n_h):
                        pss = pp.tile([128, score_fd], F32, tag="ps")
                        for j in range(score_fd // QC):
                            qs = slice(h * score_fd + j * QC,
                                       h * score_fd + (j + 1) * QC)
                            nc.tensor.matmul(pss[:, j * QC:(j + 1) * QC],
                                             lhs, q[:, qs], start=True, stop=True)
                        nc.scalar.activation(
                            e[:, kt, h * score_fd:(h + 1) * score_fd], pss[:],
                            AF.Exp,
                            accum_out=z2[:, kt, h:h + 1] if exp_accum else None)
                if not exp_accum and 'attn' not in ablate and 'qkv' not in ablate:
                    # rowsum on DVE instead (timing experiment / fallback)
                    for kt in range(KT):
                        nc.vector.reduce_sum(z2[:, kt, 0:1],
                                             e[:, kt, :].rearrange("p (a b) -> p a b", a=1),
                                             axis=mybir.AxisListType.XY)
                        nc.gpsimd.memset(z2[:, kt, 1:2], 0.0)

                # ---- V' = V / Z (bf16)
                if score_fd == 1024 and exp_accum:
                    nc.vector.tensor_add(z2[:, :, 0], z2[:, :, 0], z2[:, :, 1])
                    nc.vector.tensor_add(z2[:, :, 2], z2[:, :, 2], z2[:, :, 3])
                    nc.vector.tensor_add(zr[:, :], z2[:, :, 0], z2[:, :, 2])
                elif score_fd == 4096 and exp_accum:
                    nc.vector.tensor_copy(zr[:, :], z2[:, :, 0])
                else:
                    nc.vector.tensor_add(zr[:, :], z2[:, :, 0], z2[:, :, 1])
                nc.vector.reciprocal(zr[:, :], zr[:, :])
                for kt in range(KT):
                    nc.vector.tensor_scalar_mul(vp[:, kt, :], v[:, kt, :],
                                                zr[:, kt:kt + 1])

                # ---- conv3x3x3 + Y2, emitted in two chunks: chunk 0 fills
                # the PE gap while DVE finishes Z->R->V'; chunk 1 fills the
                # ReduceScatter window.
                c3 = dp.tile([Cy, KC], F32, tag="c3")
                y2 = dp.tile([Cs, KC], F32, tag="y2")
                if 'conv' in ablate:
                    nc.gpsimd.memset(c3[:], 0.1)

                def conv_chunk(ci):
                    if 'conv' in ablate:
                        return
                    psc = pp.tile([Cy, QC], F32, tag="ps")
                    for t in range(27):
                        kd, r = divmod(t, 9)
                        kh, kw = divmod(r, 3)
                        nc.tensor.matmul(
                            psc[:],
                            w3[:, t, :],
                            yslab[:, 2 * ci + kd:2 * ci + kd + 2,
                                  kh:kh + 16, kw:kw + 16],
                            start=(t == 0), stop=(t == 26))
                    nc.vector.tensor_copy(c3[:, ci * QC:(ci + 1) * QC], psc[:])
                    sl = slice(ci * QC, (ci + 1) * QC)
                    psy = pp.tile([Cs, QC], F32, tag="ps")
                    nc.tensor.matmul(psy[:], wy2[:], c3[:, sl], start=True, stop=True)
                    nc.scalar.activation(y2[:, sl], psy[:], AF.Relu, bias=bias[:, 3:4])
                    nc.sync.dma_start(OUT.ap()[Cs:2 * Cs, sl], y2[:, sl])

                conv_chunk(0)

                # ---- x^T partials -> DRAM collective input
                CDT = BF16 if rs_bf16 else F32
                cin = dram.tile([G * Cs, KC], CDT, tag="cin")
                cout = dram.tile([Cs, KC], CDT, tag="cout")
                for ci in range(N // QC):
                    psx = pp.tile([Cs, QC], F32, tag="ps")
                    for kt in range(KT):
                        nc.tensor.matmul(psx[:], vp[:, kt, :],
                                         e[:, kt, ci * QC:(ci + 1) * QC],
                                         start=(kt == 0), stop=(kt == KT - 1))
                    xc = chp.tile([Cs, QC], CDT, tag="xc")
                    nc.vector.tensor_copy(xc[:], psx[:])
                    gg, half = divmod(ci, 2)
                    nc.sync.dma_start(
                        cin[gg * Cs:(gg + 1) * Cs, half * QC:(half + 1) * QC], xc[:])

                if 'rs' not in ablate:
                    nc.gpsimd.collective_compute(
                        "ReduceScatter", mybir.AluOpType.add,
                        replica_groups=[[0, 1, 2, 3], [4, 5, 6, 7]],
                        ins=[cin[:]], outs=[cout[:]],
                    )
                else:
                    nc.sync.dma_start(cout[:], cin[0:Cs, :])

                conv_chunk(1)

                # ---- post-RS: out-projection, mul by S+pe
                xr = dp.tile([Cs, KC], CDT, tag="xr")
                nc.sync.dma_start(xr[:], cout[:])
                wo_c = wo
                if rs_bf16:
                    wo_c = dp.tile([Cs, Cs], BF16, tag="wo_b")
                    nc.vector.tensor_copy(wo_c[:], wo[:])
                zc = dp.tile([Cs, KC], F32, tag="zc")
                zo = dp.tile([Cs, KC], F32, tag="zo")
                for ci in range(2):
                    sl = slice(ci * QC, (ci + 1) * QC)
                    psz = pp.tile([Cs, QC], F32, tag="ps")
                    nc.tensor.matmul(psz[:], wo_c[:], xr[:, sl], start=True, stop=True)
                    nc.scalar.activation(zc[:, sl], psz[:], AF.Relu, bias=bias[:, 2:3])
                nc.vector.tensor_mul(zo[:], zc[:], sk[:])
                nc.sync.dma_start(OUT.ap()[0:Cs, :], zo[:])

    nc.compile()
    return nc


class _Runner:
    """Builds the bass module once and a single reusable jitted callable
    (re-jitting per call would re-trace + re-hash the BIR module: ~600ms)."""

    def __init__(self, repeat=1, ablate=(), **kw):
        install_neuronx_cc_hook()
        nc = _build(repeat, ablate, **kw)
        pid = nc.partition_id_tensor.name if nc.partition_id_tensor else None
        in_names, out_names, out_avals = [], [], []
        for alloc in nc.m.functions[0].allocations:
            if not isinstance(alloc, mybir.MemoryLocationSet):
                continue
            name = alloc.memorylocations[0].name
            if alloc.kind == "ExternalInput":
                if name != pid:
                    in_names.append(name)
            elif alloc.kind == "ExternalOutput":
                out_names.append(name)
                out_avals.append(jax.core.ShapedArray(
                    tuple(alloc.tensor_shape), mybir.dt.np(alloc.dtype)))
        self.in_names, self.out_names, self.out_avals = in_names, out_names, out_avals
        all_names = in_names + out_names + ([pid] if pid else [])

        def _body(*args):
            operands = list(args)
            if pid is not None:
                operands.append(bass2jax.partition_id_tensor())
            return tuple(_bass_exec_p.bind(
                *operands, out_avals=tuple(out_avals), in_names=tuple(all_names),
                out_names=tuple(out_names), lowering_input_output_aliases=(),
                sim_require_finite=True, sim_require_nnan=True, nc=nc))

        mesh = self.mesh = Mesh(np.asarray(jax.devices()[:NC]), ("core",))
        sp = (PartitionSpec("core"),)
        n_in = len(in_names) + len(out_names)
        self.fn = jax.jit(
            shard_map(_body, mesh=mesh, in_specs=sp * n_in,
                      out_specs=sp * len(out_names), check_rep=False),
            keep_unused=True)

    def device_args(self, in_maps):
        """Pre-stage all inputs on device (sharded) for low-overhead timed calls."""
        from jax.sharding import NamedSharding
        sh = NamedSharding(self.mesh, PartitionSpec("core"))
        cat = [np.concatenate([in_maps[c][n] for c in range(NC)], axis=0)
               for n in self.in_names]
        zz = [np.zeros((NC * a.shape[0], *a.shape[1:]), a.dtype)
              for a in self.out_avals]
        return [jax.device_put(a, sh) for a in cat + zz]

    def __call__(self, in_maps):
        outs = self.fn(*self.device_args(in_maps))
        jax.block_until_ready(outs)
        return [
            {n: np.asarray(outs[i]).reshape(NC, *self.out_avals[i].shape)[c]
             for i, n in enumerate(self.out_names)}
            for c in range(NC)
        ]


def _get(repeat=1, ablate=(), **kw):
    key = (repeat, tuple(sorted(ablate)), tuple(sorted(kw.items())))
    if key not in _cache:
        _cache[key] = _Runner(repeat, ablate, **kw)
    return _cache[key]


# ---------------------------------------------------------------- entry point
def kernel(**inputs):
    in_maps = _prepare(inputs)
    results = _get(1)(in_maps)
    out = np.zeros((B, 2 * Cs, D, H, W), np.float32)
    for core in range(NC):
        b, g = divmod(core, G)
        blk = results[core]["OUT"].reshape(2 * Cs, D // G, H, W)
        out[b, :, g * (D // G):(g + 1) * (D // G)] = blk
    return out
